# revision 12
# baseline (speedup 1.0000x reference)
"""Trainium2 Bass kernel for nn_BrainRegion (liquid-gated recurrent cell).

Computes, for full inputs (B=8192, IN=H=2048):
    xin  = concat([x_t, state], -1)
    cand = tanh(xin @ Wc + state @ Uc + bc)
    gate = sigmoid(xin @ Wg + state @ Ug + bg)
    alpha = exp(-1/exp(log_step))
    h    = alpha * state + (1 - alpha) * gate * cand
    out  = layernorm(h) * gamma + beta

Strategy: data-parallel over batch across 8 NeuronCores (1024 rows/core),
weights replicated.  Algebraic fold: xin@Wc + state@Uc == x_t@Wc[:IN] +
state@(Wc[IN:] + Uc), which removes one third of the FLOPs.  Mixed
precision on the TensorEngine: the x_t-side matmuls and the first 512
state channels run in fp8 e4m3 with perf_mode=DoubleRow (256-deep
contraction per instruction, issuing at the same 216ns as a 512-wide
bf16 matmul = 2x throughput); the remaining state-side channels run in
bf16 (the folded state weights are 3x larger in variance, so full fp8
there would blow the error budget).  Scale folding keeps the shared
PSUM accumulation consistent: activations*16 and W*256 in fp8,
Ws*4096 in bf16 (all powers of two, exact); the epilogue activations
fold 1/4096 back in.  Loop order is j-outer / group-inner so every
weight byte is DMAed exactly once; startup DMAs are emitted in exact
first-phase consumption order; the last j slice runs single-group
phases to minimize the post-matmul tail.
"""

import sys

if "/opt/trn_rl_repo" not in sys.path:
    sys.path.insert(0, "/opt/trn_rl_repo")

import numpy as np
import ml_dtypes

B, IN, H = 8192, 2048, 2048
NCORES = 8
BC = B // NCORES      # rows per core (1024)
P = 128               # partitions
G = BC // P           # batch groups per core (8)
GP = G // 2           # group pairs (4)
NJ = 4                # H slices
NSL = H // NJ         # slice width (512)
K8 = H // 256         # fp8 DoubleRow k-chunks on the x side (8)
SK8 = 2               # state-side fp8 DoubleRow k-chunks (first 512 ch)
SCUT = SK8 * 256      # state channels handled in fp8 (512)
K16 = (H - SCUT) // 128   # bf16 state k-chunks (12)
KQ = 4                # bf16 k-chunks per quarter weight tile
NQ = K16 // KQ        # quarter tiles (3)
EPS = 1e-5
SX = 16.0             # fp8 activation scale
SW8 = 256.0           # fp8 weight scale
SW16 = 4096.0         # bf16 state-weight scale (= SX*SW8, exact pow2)
RS = 1.0 / 4096.0     # epilogue rescale

bf16 = ml_dtypes.bfloat16
e4m3 = ml_dtypes.float8_e4m3

# Set by test.py to collect a hardware profile.
TRACE = False
LAST_RESULTS = None

_compiled = {}


def _build(flags):
    """Trace + compile the SPMD device program. flags = (has_bc, has_bg,
    has_gamma, has_beta) selects optional elementwise passes."""
    from contextlib import ExitStack

    import concourse.bass as bass
    import concourse.tile as tile
    from concourse import bacc, mybir

    has_bc, has_bg, has_gamma, has_beta = flags
    f32 = mybir.dt.float32
    bft = mybir.dt.bfloat16
    f8 = mybir.dt.float8e4
    AF = mybir.ActivationFunctionType
    OP = mybir.AluOpType
    DR = mybir.MatmulPerfMode.DoubleRow

    nc = bacc.Bacc("TRN2", target_bir_lowering=False, debug=False,
                   num_devices=NCORES)

    # DRAM I/O. Everything is pre-arranged on host so every DMA below is
    # contiguous (partition dim first):
    #   x8:    [G, P, K8, 2, P]    e4m3, [g,p,k,i,m] = 16*x[g*128+m, k*256+i*128+p]
    #   s8:    [G, P, SK8, 2, P]   e4m3, same layout for state channels < SCUT
    #   s16:   [G, P, K16, P]      bf16, [g,p,k,m] = s[g*128+m, SCUT+k*128+p]
    #   w*x8:  [NJ, K8, P, 2, NSL] e4m3, [j,k,p,i,n] = 256*W[k*256+i*128+p, j*512+n]
    #   w*s8:  [NJ, SK8, P, 2, NSL] e4m3, same for state-weight rows < SCUT
    #   w*s16: [NJ, P, K16, NSL]   bf16, [j,p,k,n] = 4096*W[SCUT+k*128+p, j*512+n]
    x8 = nc.dram_tensor("x8", [G, P, K8, 2, P], f8, kind="ExternalInput").ap()
    s8 = nc.dram_tensor("s8", [G, P, SK8, 2, P], f8,
                        kind="ExternalInput").ap()
    s16 = nc.dram_tensor("s16", [G, P, K16, P], bft,
                         kind="ExternalInput").ap()
    stb = nc.dram_tensor("stb", [BC, H], bft, kind="ExternalInput").ap()
    wcx8 = nc.dram_tensor("wcx8", [NJ, K8, P, 2, NSL], f8,
                          kind="ExternalInput").ap()
    wgx8 = nc.dram_tensor("wgx8", [NJ, K8, P, 2, NSL], f8,
                          kind="ExternalInput").ap()
    wcs8 = nc.dram_tensor("wcs8", [NJ, SK8, P, 2, NSL], f8,
                          kind="ExternalInput").ap()
    wgs8 = nc.dram_tensor("wgs8", [NJ, SK8, P, 2, NSL], f8,
                          kind="ExternalInput").ap()
    wcs16 = nc.dram_tensor("wcs16", [NJ, P, K16, NSL], bft,
                           kind="ExternalInput").ap()
    wgs16 = nc.dram_tensor("wgs16", [NJ, P, K16, NSL], bft,
                           kind="ExternalInput").ap()
    logb = nc.dram_tensor("logb", [P, H], f32, kind="ExternalInput").ap()
    vecs = {}
    for name, used in (("bcb", has_bc), ("bgb", has_bg),
                       ("gammab", has_gamma), ("betab", has_beta)):
        if used:
            vecs[name] = nc.dram_tensor(name, [P, H], f32,
                                        kind="ExternalInput").ap()
    out = nc.dram_tensor("out", [BC, H], f32, kind="ExternalOutput").ap()

    with tile.TileContext(nc) as tc, ExitStack() as ctx:
        singles = ctx.enter_context(tc.tile_pool(name="singles", bufs=1))
        actp = ctx.enter_context(tc.tile_pool(name="actp", bufs=1))
        wxp = ctx.enter_context(tc.tile_pool(name="wxp", bufs=2))
        wsp = ctx.enter_context(tc.tile_pool(name="wsp", bufs=2))
        psp = ctx.enter_context(tc.tile_pool(name="psp", bufs=2, space="PSUM"))
        epp = ctx.enter_context(tc.tile_pool(name="epp", bufs=2))
        stp = ctx.enter_context(tc.tile_pool(name="stp", bufs=2))
        hp = ctx.enter_context(tc.tile_pool(name="hp", bufs=1))
        statp = ctx.enter_context(tc.tile_pool(name="statp", bufs=1))
        normp = ctx.enter_context(tc.tile_pool(name="normp", bufs=4))
        outp = ctx.enter_context(tc.tile_pool(name="outp", bufs=3))

        wx_t = {"c": {}, "g": {}}   # (j, k) -> [P, 2, NSL] f8, x side
        wsd_t = {"c": {}, "g": {}}  # (j, k) -> [P, 2, NSL] f8, state side
        ws_t = {}                   # (j, mat, q) -> [P, KQ, NSL] bf16
        act_x = {}
        act_s8 = {}
        act_s = {}

        def load_wx(j):
            for k in range(K8):
                for mat, dram in (("c", wcx8), ("g", wgx8)):
                    t = wxp.tile([P, 2, NSL], f8, name=f"wx{mat}_{j}_{k}",
                                 tag=f"wx{mat}_{k}")
                    nc.sync.dma_start(out=t[:], in_=dram[j, k])
                    wx_t[mat][(j, k)] = t

        def load_wsd(j):
            for k in range(SK8):
                for mat, dram in (("c", wcs8), ("g", wgs8)):
                    t = wxp.tile([P, 2, NSL], f8, name=f"wsd{mat}_{j}_{k}",
                                 tag=f"wsd{mat}_{k}")
                    nc.sync.dma_start(out=t[:], in_=dram[j, k])
                    wsd_t[mat][(j, k)] = t

        def load_ws(j):
            for q in range(NQ):
                for mat, dram in (("c", wcs16), ("g", wgs16)):
                    t = wsp.tile([P, KQ, NSL], bft,
                                 name=f"ws{mat}_{j}_{q}", tag=f"ws{mat}{q}")
                    nc.sync.dma_start(
                        out=t[:], in_=dram[j][:, q * KQ:(q + 1) * KQ, :])
                    ws_t[(j, mat, q)] = t

        def load_x(g):
            t = actp.tile([P, K8, 2, P], f8, name=f"x8_{g}", tag=f"x{g}")
            nc.sync.dma_start(out=t[:], in_=x8[g])
            act_x[g] = t

        def load_s8(g):
            t = actp.tile([P, SK8, 2, P], f8, name=f"s8_{g}", tag=f"d{g}")
            nc.sync.dma_start(out=t[:], in_=s8[g])
            act_s8[g] = t

        def load_s(g):
            t = actp.tile([P, K16, P], bft, name=f"s16_{g}", tag=f"s{g}")
            nc.sync.dma_start(out=t[:], in_=s16[g])
            act_s[g] = t

        # ---- startup DMAs, ordered to match first-phase consumption ----
        load_x(0)
        load_x(1)
        load_wx(0)
        load_s8(0)
        load_s8(1)
        load_wsd(0)
        load_s(0)
        load_s(1)
        load_ws(0)
        for g in range(2, G):
            load_x(g)
            load_s8(g)
            load_s(g)

        # alpha = exp(-exp(-log_step)) as a bf16 [P, H] broadcast, computed
        # in NSL chunks through the outp ring (before any output use).
        alpha_t = singles.tile([P, H], bft, name="alpha_t")
        for q in range(NJ):
            qsl = slice(q * NSL, (q + 1) * NSL)
            t = outp.tile([P, NSL], f32, name=f"lg_{q}", tag="ot")
            nc.sync.dma_start(out=t[:], in_=logb[:, qsl])
            nc.scalar.activation(t[:], t[:], AF.Exp, scale=-1.0)
            nc.scalar.activation(alpha_t[:, qsl], t[:], AF.Exp, scale=-1.0)
        eps_t = singles.tile([P, 1], f32, name="eps_t")
        nc.vector.memset(eps_t[:], EPS)
        vt = {}
        for name in vecs:
            vt[name] = singles.tile([P, H], f32, name=name + "_t")
            nc.sync.dma_start(out=vt[name][:], in_=vecs[name][:])

        h_t = {}
        stats_t = {}

        def ensure_group_tiles(groups):
            for g in groups:
                if g not in h_t:
                    h_t[g] = hp.tile([P, H], bft, name=f"h_{g}", tag=f"h{g}")
                    stats_t[g] = statp.tile([P, NJ, 6], f32,
                                            name=f"stats_{g}", tag=f"st{g}")

        def phase(j, groups, pid):
            """One PSUM phase: full H-slice j accumulation + epilogue for
            the given batch groups (1 or 2 of them)."""
            jsl = slice(j * NSL, (j + 1) * NSL)
            ensure_group_tiles(groups)

            st_t = []
            for gi, g in enumerate(groups):
                t = stp.tile([P, NSL], bft, name=f"st_{j}_{g}",
                             tag=f"st{gi}")
                nc.sync.dma_start(out=t[:], in_=stb[g * P:(g + 1) * P, jsl])
                st_t.append(t)

            # PSUM: tags pc0/pc1/pg0/pg1 x bufs=2 = all 8 banks.  A 4-group
            # phase occupies both ring slots of each tag simultaneously.
            pc = [psp.tile([P, NSL], f32, name=f"pc_{j}_{pid}_{gi}",
                           tag=f"pc{gi % 2}") for gi in range(len(groups))]
            pg = [psp.tile([P, NSL], f32, name=f"pg_{j}_{pid}_{gi}",
                           tag=f"pg{gi % 2}") for gi in range(len(groups))]

            # fp8 DoubleRow x-side accumulation (k = 256 per MM)
            for k in range(K8):
                for gi, g in enumerate(groups):
                    xk = act_x[g][:, k]
                    nc.tensor.matmul(pc[gi][:], xk, wx_t["c"][(j, k)][:],
                                     start=(k == 0), stop=False, perf_mode=DR)
                    nc.tensor.matmul(pg[gi][:], xk, wx_t["g"][(j, k)][:],
                                     start=(k == 0), stop=False, perf_mode=DR)
            # fp8 DoubleRow state-side accumulation (first SCUT channels)
            for k in range(SK8):
                for gi, g in enumerate(groups):
                    sk = act_s8[g][:, k]
                    nc.tensor.matmul(pc[gi][:], sk, wsd_t["c"][(j, k)][:],
                                     start=False, stop=False, perf_mode=DR)
                    nc.tensor.matmul(pg[gi][:], sk, wsd_t["g"][(j, k)][:],
                                     start=False, stop=False, perf_mode=DR)
            # bf16 state-side accumulation (k = 128 per MM)
            for k in range(K16):
                q, kk = divmod(k, KQ)
                wc = ws_t[(j, "c", q)]
                wg = ws_t[(j, "g", q)]
                for gi, g in enumerate(groups):
                    sk = act_s[g][:, k, :]
                    nc.tensor.matmul(pc[gi][:], sk, wc[:, kk, :],
                                     start=False, stop=(k == K16 - 1))
                    nc.tensor.matmul(pg[gi][:], sk, wg[:, kk, :],
                                     start=False, stop=(k == K16 - 1))

            # epilogue for this (j, groups) slice
            for gi, g in enumerate(groups):
                sc = epp.tile([P, NSL], f32, name=f"sc_{j}_{pid}_{gi}",
                              tag="sc")
                sg = epp.tile([P, NSL], f32, name=f"sg_{j}_{pid}_{gi}",
                              tag="sg")
                if has_bc:
                    nc.vector.scalar_tensor_tensor(
                        sc[:], pc[gi][:], RS, vt["bcb"][:, jsl],
                        op0=OP.mult, op1=OP.add)
                    nc.scalar.activation(sc[:], sc[:], AF.Tanh)
                else:
                    nc.scalar.activation(sc[:], pc[gi][:], AF.Tanh, scale=RS)
                if has_bg:
                    nc.vector.scalar_tensor_tensor(
                        sg[:], pg[gi][:], RS, vt["bgb"][:, jsl],
                        op0=OP.mult, op1=OP.add)
                    nc.scalar.activation(sg[:], sg[:], AF.Sigmoid)
                else:
                    nc.scalar.activation(sg[:], pg[gi][:], AF.Sigmoid,
                                         scale=RS)

                # h = gc + alpha*(state - gc), with gc = gate*cand
                t2 = epp.tile([P, NSL], f32, name=f"t2_{j}_{pid}_{gi}",
                              tag="t2")
                nc.vector.tensor_mul(t2[:], sc[:], sg[:])
                nc.vector.tensor_sub(sc[:], st_t[gi][:], t2[:])
                nc.vector.tensor_mul(sc[:], sc[:], alpha_t[:, jsl])
                nc.vector.tensor_add(t2[:], t2[:], sc[:])

                nc.vector.bn_stats(out=stats_t[g][:, j, :], in_=t2[:])
                nc.vector.tensor_copy(out=h_t[g][:, jsl], in_=t2[:])

                if j == NJ - 1:
                    # layernorm + output for this group
                    mv = normp.tile([P, 2], f32, name=f"mv_{g}", tag="mv")
                    nc.vector.bn_aggr(out=mv[:], in_=stats_t[g][:])
                    rstd = normp.tile([P, 1], f32, name=f"rstd_{g}",
                                      tag="rstd")
                    nc.scalar.activation(rstd[:], mv[:, 1:2], AF.Sqrt,
                                         bias=eps_t[:])
                    nc.vector.reciprocal(rstd[:], rstd[:])
                    for q in range(NJ):
                        hs = slice(q * NSL, (q + 1) * NSL)
                        ot = outp.tile([P, NSL], f32,
                                       name=f"ot_{g}_{q}", tag="ot")
                        nc.vector.tensor_scalar(ot[:], h_t[g][:, hs],
                                                mv[:, 0:1], rstd[:],
                                                op0=OP.subtract, op1=OP.mult)
                        if has_gamma:
                            nc.vector.tensor_mul(ot[:], ot[:],
                                                 vt["gammab"][:, hs])
                        if has_beta:
                            nc.vector.tensor_add(ot[:], ot[:],
                                                 vt["betab"][:, hs])
                        nc.sync.dma_start(out=out[g * P:(g + 1) * P, hs],
                                          in_=ot[:])

        # ---- main loops: j = H slice (outer), batch groups inner.
        # First j slice runs 4-group phases (38us of matmul per phase)
        # to cover the initial weight-streaming burst; the last j slice
        # runs single-group phases to shorten the post-matmul tail.
        for j in range(NJ):
            if j == 0:
                group_sets = [(0, 1, 2, 3), (4, 5, 6, 7)]
            elif j < NJ - 1:
                group_sets = [(2 * gp, 2 * gp + 1) for gp in range(GP)]
            else:
                group_sets = [(g,) for g in range(G)]
            for pid, groups in enumerate(group_sets):
                phase(j, groups, pid)
                # prefetch next j's weights while this super-phase runs
                if pid == 0 and j + 1 < NJ:
                    load_wx(j + 1)
                    load_wsd(j + 1)
                    load_ws(j + 1)

    nc.compile()
    return nc


def _get_compiled(flags):
    if flags not in _compiled:
        _compiled[flags] = _build(flags)
    return _compiled[flags]


def kernel(x_t, state, Wc, Uc, bc, Wg, Ug, bg, log_step, gamma, beta):
    global LAST_RESULTS
    from concourse import bass_utils

    x_t = np.asarray(x_t, np.float32)
    state = np.asarray(state, np.float32)
    Wc = np.asarray(Wc, np.float32)
    Uc = np.asarray(Uc, np.float32)
    Wg = np.asarray(Wg, np.float32)
    Ug = np.asarray(Ug, np.float32)
    bc = np.asarray(bc, np.float32)
    bg = np.asarray(bg, np.float32)
    log_step = np.asarray(log_step, np.float32)
    gamma = np.asarray(gamma, np.float32)
    beta = np.asarray(beta, np.float32)

    # fold the recurrent weights and pre-tile for the device
    def w8tile(w, nk):  # [j,k,p,i,n] = 256*W[k*256+i*128+p, j*512+n], e4m3
        a = np.clip(w * SW8, -240.0, 240.0).astype(e4m3)
        return np.ascontiguousarray(
            a.reshape(nk, 2, P, NJ, NSL).transpose(3, 0, 2, 1, 4))

    def ws16tile(w):  # [j,p,k,n] = 4096*W[k*128+p, j*512+n], bf16
        a = (w * SW16).astype(bf16)
        return np.ascontiguousarray(
            a.reshape(K16, P, NJ, NSL).transpose(2, 1, 0, 3))

    Wcs = Wc[IN:] + Uc
    Wgs = Wg[IN:] + Ug
    w_maps = {
        "wcx8": w8tile(Wc[:IN], K8),
        "wgx8": w8tile(Wg[:IN], K8),
        "wcs8": w8tile(Wcs[:SCUT], SK8),
        "wgs8": w8tile(Wgs[:SCUT], SK8),
        "wcs16": ws16tile(Wcs[SCUT:]),
        "wgs16": ws16tile(Wgs[SCUT:]),
    }
    logb = np.ascontiguousarray(
        np.broadcast_to(log_step.reshape(1, H), (P, H)))

    flags = (bool(bc.any()), bool(bg.any()),
             bool((gamma != 1.0).any()), bool(beta.any()))
    vec_maps = {}
    if flags[0]:
        vec_maps["bcb"] = np.ascontiguousarray(
            np.broadcast_to(bc.reshape(1, H), (P, H)))
    if flags[1]:
        vec_maps["bgb"] = np.ascontiguousarray(
            np.broadcast_to(bg.reshape(1, H), (P, H)))
    if flags[2]:
        vec_maps["gammab"] = np.ascontiguousarray(
            np.broadcast_to(gamma.reshape(1, H), (P, H)))
    if flags[3]:
        vec_maps["betab"] = np.ascontiguousarray(
            np.broadcast_to(beta.reshape(1, H), (P, H)))

    nc = _get_compiled(flags)

    # per-core activation shards, pre-tiled
    def a8tile(a, nk):  # [g,p,k,i,m] = 16*a[g*128+m, k*256+i*128+p], e4m3
        q = np.clip(a * SX, -240.0, 240.0).astype(e4m3)
        return np.ascontiguousarray(
            q.reshape(G, P, nk, 2, P).transpose(0, 4, 2, 3, 1))

    def s16tile(a):  # [g,p,k,m] = a[g*128+m, k*128+p], bf16
        return np.ascontiguousarray(
            a.astype(bf16).reshape(G, P, K16, P).transpose(0, 3, 2, 1))

    in_maps = []
    for c in range(NCORES):
        rows = slice(c * BC, (c + 1) * BC)
        sr = state[rows]
        m = {
            "x8": a8tile(x_t[rows], K8),
            "s8": a8tile(sr[:, :SCUT], SK8),
            "s16": s16tile(sr[:, SCUT:]),
            "stb": np.ascontiguousarray(sr.astype(bf16)),
            "logb": logb,
        }
        m.update(w_maps)
        m.update(vec_maps)
        in_maps.append(m)

    trace_kwargs = {}
    if TRACE:
        trace_kwargs["trace_cores"] = list(range(NCORES))
    res = bass_utils.run_bass_kernel_spmd(
        nc, in_maps, core_ids=list(range(NCORES)), trace=TRACE,
        **trace_kwargs)
    LAST_RESULTS = res
    return np.concatenate([res.results[c]["out"] for c in range(NCORES)],
                          axis=0)


# revision 21
# speedup vs baseline: 1.0276x; 1.0276x over previous
"""Trainium2 Bass kernel for nn_BrainRegion (liquid-gated recurrent cell).

Computes, for full inputs (B=8192, IN=H=2048):
    xin  = concat([x_t, state], -1)
    cand = tanh(xin @ Wc + state @ Uc + bc)
    gate = sigmoid(xin @ Wg + state @ Ug + bg)
    alpha = exp(-1/exp(log_step))
    h    = alpha * state + (1 - alpha) * gate * cand
    out  = layernorm(h) * gamma + beta

Strategy: data-parallel over batch across 8 NeuronCores (1024 rows/core),
weights replicated.  Algebraic fold: xin@Wc + state@Uc == x_t@Wc[:IN] +
state@(Wc[IN:] + Uc), which removes one third of the FLOPs.  Mixed
precision on the TensorEngine: the x_t-side matmuls and the first 512
state channels run in fp8 e4m3 with perf_mode=DoubleRow (256-deep
contraction per instruction, issuing at the same 216ns as a 512-wide
bf16 matmul = 2x throughput); the remaining state-side channels run in
bf16 (the folded state weights are 3x larger in variance, so full fp8
there would blow the error budget).  Scale folding keeps the shared
PSUM accumulation consistent: activations*16 and W*256 in fp8,
Ws*4096 in bf16 (all powers of two, exact); the epilogue activations
fold 1/4096 back in.  Loop order is j-outer / group-inner so every
weight byte is DMAed exactly once; startup DMAs are emitted in exact
first-phase consumption order; the last j slice runs single-group
phases to minimize the post-matmul tail.
"""

import sys

if "/opt/trn_rl_repo" not in sys.path:
    sys.path.insert(0, "/opt/trn_rl_repo")

import numpy as np
import ml_dtypes

B, IN, H = 8192, 2048, 2048
NCORES = 8
BC = B // NCORES      # rows per core (1024)
P = 128               # partitions
G = BC // P           # batch groups per core (8)
GP = G // 2           # group pairs (4)
NJ = 4                # H slices
NSL = H // NJ         # slice width (512)
K8 = H // 256         # fp8 DoubleRow k-chunks on the x side (8)
SK8 = 2               # state-side fp8 DoubleRow k-chunks (first 512 ch)
SCUT = SK8 * 256      # state channels handled in fp8 (512)
K16 = (H - SCUT) // 128   # bf16 state k-chunks (12)
KQ = 4                # bf16 k-chunks per quarter weight tile
NQ = K16 // KQ        # quarter tiles (3)
EPS = 1e-5
SX = 16.0             # fp8 activation scale
SW8 = 256.0           # fp8 weight scale
SW16 = 4096.0         # bf16 state-weight scale (= SX*SW8, exact pow2)
RS = 1.0 / 4096.0     # epilogue rescale

bf16 = ml_dtypes.bfloat16
e4m3 = ml_dtypes.float8_e4m3

# Set by test.py to collect a hardware profile.
TRACE = False
LAST_RESULTS = None

_compiled = {}


def _build(flags):
    """Trace + compile the SPMD device program. flags = (has_bc, has_bg,
    has_gamma, has_beta) selects optional elementwise passes."""
    from contextlib import ExitStack

    import concourse.bass as bass
    import concourse.tile as tile
    from concourse import bacc, mybir

    has_bc, has_bg, has_gamma, has_beta = flags
    f32 = mybir.dt.float32
    bft = mybir.dt.bfloat16
    f8 = mybir.dt.float8e4
    AF = mybir.ActivationFunctionType
    OP = mybir.AluOpType
    DR = mybir.MatmulPerfMode.DoubleRow

    nc = bacc.Bacc("TRN2", target_bir_lowering=False, debug=False,
                   num_devices=NCORES)

    # DRAM I/O. Everything is pre-arranged on host so every DMA below is
    # contiguous (partition dim first):
    #   x8:    [G, P, K8, 2, P]    e4m3, [g,p,k,i,m] = 16*x[g*128+m, k*256+i*128+p]
    #   s8:    [G, P, SK8, 2, P]   e4m3, same layout for state channels < SCUT
    #   s16:   [G, P, K16, P]      bf16, [g,p,k,m] = s[g*128+m, SCUT+k*128+p]
    #   w*x8:  [NJ, K8, P, 2, NSL] e4m3, [j,k,p,i,n] = 256*W[k*256+i*128+p, j*512+n]
    #   w*s8:  [NJ, SK8, P, 2, NSL] e4m3, same for state-weight rows < SCUT
    #   w*s16: [NJ, P, K16, NSL]   bf16, [j,p,k,n] = 4096*W[SCUT+k*128+p, j*512+n]
    x8 = nc.dram_tensor("x8", [G, P, K8, 2, P], f8, kind="ExternalInput").ap()
    s8 = nc.dram_tensor("s8", [G, P, SK8, 2, P], f8,
                        kind="ExternalInput").ap()
    s16 = nc.dram_tensor("s16", [G, P, K16, P], bft,
                         kind="ExternalInput").ap()
    stb = nc.dram_tensor("stb", [BC, H], bft, kind="ExternalInput").ap()
    wcx8 = nc.dram_tensor("wcx8", [NJ, K8, P, 2, NSL], f8,
                          kind="ExternalInput").ap()
    wgx8 = nc.dram_tensor("wgx8", [NJ, K8, P, 2, NSL], f8,
                          kind="ExternalInput").ap()
    wcs8 = nc.dram_tensor("wcs8", [NJ, SK8, P, 2, NSL], f8,
                          kind="ExternalInput").ap()
    wgs8 = nc.dram_tensor("wgs8", [NJ, SK8, P, 2, NSL], f8,
                          kind="ExternalInput").ap()
    wcs16 = nc.dram_tensor("wcs16", [NJ, P, K16, NSL], bft,
                           kind="ExternalInput").ap()
    wgs16 = nc.dram_tensor("wgs16", [NJ, P, K16, NSL], bft,
                           kind="ExternalInput").ap()
    logb = nc.dram_tensor("logb", [P, H], f32, kind="ExternalInput").ap()
    vecs = {}
    for name, used in (("bcb", has_bc), ("bgb", has_bg),
                       ("gammab", has_gamma), ("betab", has_beta)):
        if used:
            vecs[name] = nc.dram_tensor(name, [P, H], f32,
                                        kind="ExternalInput").ap()
    out = nc.dram_tensor("out", [BC, H], f32, kind="ExternalOutput").ap()

    with tile.TileContext(nc) as tc, ExitStack() as ctx:
        singles = ctx.enter_context(tc.tile_pool(name="singles", bufs=1))
        actp = ctx.enter_context(tc.tile_pool(name="actp", bufs=1))
        wxp = ctx.enter_context(tc.tile_pool(name="wxp", bufs=2))
        wsp = ctx.enter_context(tc.tile_pool(name="wsp", bufs=2))
        psp = ctx.enter_context(tc.tile_pool(name="psp", bufs=2, space="PSUM"))
        epp = ctx.enter_context(tc.tile_pool(name="epp", bufs=2))
        stp = ctx.enter_context(tc.tile_pool(name="stp", bufs=2))
        hp = ctx.enter_context(tc.tile_pool(name="hp", bufs=1))
        statp = ctx.enter_context(tc.tile_pool(name="statp", bufs=1))
        normp = ctx.enter_context(tc.tile_pool(name="normp", bufs=4))
        outp = ctx.enter_context(tc.tile_pool(name="outp", bufs=3))

        wx_t = {"c": {}, "g": {}}   # (j, k) -> [P, 2, NSL] f8, x side
        wsd_t = {"c": {}, "g": {}}  # (j, k) -> [P, 2, NSL] f8, state side
        ws_t = {}                   # (j, mat, q) -> [P, KQ, NSL] bf16
        act_x = {}
        act_s8 = {}
        act_s = {}

        def load_wx(j):
            for k in range(K8):
                for mat, dram in (("c", wcx8), ("g", wgx8)):
                    t = wxp.tile([P, 2, NSL], f8, name=f"wx{mat}_{j}_{k}",
                                 tag=f"wx{mat}_{k}")
                    nc.sync.dma_start(out=t[:], in_=dram[j, k])
                    wx_t[mat][(j, k)] = t

        def load_wsd(j):
            for k in range(SK8):
                for mat, dram in (("c", wcs8), ("g", wgs8)):
                    t = wxp.tile([P, 2, NSL], f8, name=f"wsd{mat}_{j}_{k}",
                                 tag=f"wsd{mat}_{k}")
                    nc.sync.dma_start(out=t[:], in_=dram[j, k])
                    wsd_t[mat][(j, k)] = t

        def load_ws(j):
            for q in range(NQ):
                for mat, dram in (("c", wcs16), ("g", wgs16)):
                    t = wsp.tile([P, KQ, NSL], bft,
                                 name=f"ws{mat}_{j}_{q}", tag=f"ws{mat}{q}")
                    nc.sync.dma_start(
                        out=t[:], in_=dram[j][:, q * KQ:(q + 1) * KQ, :])
                    ws_t[(j, mat, q)] = t

        def load_x(g):
            t = actp.tile([P, K8, 2, P], f8, name=f"x8_{g}", tag=f"x{g}")
            nc.sync.dma_start(out=t[:], in_=x8[g])
            act_x[g] = t

        def load_s8(g):
            t = actp.tile([P, SK8, 2, P], f8, name=f"s8_{g}", tag=f"d{g}")
            nc.sync.dma_start(out=t[:], in_=s8[g])
            act_s8[g] = t

        def load_s(g):
            t = actp.tile([P, K16, P], bft, name=f"s16_{g}", tag=f"s{g}")
            nc.sync.dma_start(out=t[:], in_=s16[g])
            act_s[g] = t

        # ---- startup DMAs: ONLY what the first (4-group) phase consumes,
        # ordered to match its consumption.  Everything else is deferred so
        # the proportional-share DMA queues don't starve the critical path.
        for g in range(4):
            load_x(g)
        load_wx(0)
        for g in range(4):
            load_s8(g)
        load_wsd(0)
        for g in range(4):
            load_s(g)
        load_ws(0)

        alpha_t = singles.tile([P, H], bft, name="alpha_t")

        def emit_alpha():
            # alpha = exp(-exp(-log_step)) as a bf16 [P, H] broadcast,
            # computed in NSL chunks through the outp ring.
            for q in range(NJ):
                qsl = slice(q * NSL, (q + 1) * NSL)
                t = outp.tile([P, NSL], f32, name=f"lg_{q}", tag="ot")
                nc.sync.dma_start(out=t[:], in_=logb[:, qsl])
                nc.scalar.activation(t[:], t[:], AF.Exp, scale=-1.0)
                nc.scalar.activation(alpha_t[:, qsl], t[:], AF.Exp,
                                     scale=-1.0)

        eps_t = singles.tile([P, 1], f32, name="eps_t")
        nc.vector.memset(eps_t[:], EPS)
        vt = {}
        for name in vecs:
            vt[name] = singles.tile([P, H], f32, name=name + "_t")
            nc.sync.dma_start(out=vt[name][:], in_=vecs[name][:])

        h_t = {}
        stats_t = {}

        def ensure_group_tiles(groups):
            for g in groups:
                if g not in h_t:
                    h_t[g] = hp.tile([P, H], bft, name=f"h_{g}", tag=f"h{g}")
                    nslot = NJ + 1 if g == G - 1 else NJ
                    stats_t[g] = statp.tile([P, nslot, 6], f32,
                                            name=f"stats_{g}", tag=f"st{g}")

        def phase(j, groups, pid, off=0, width=NSL, stat_slot=None,
                  do_ln=False, mid_cb=None):
            """One PSUM phase: H-slice j (columns [off, off+width) within
            the slice) accumulation + epilogue for the given batch groups."""
            jsl = slice(j * NSL + off, j * NSL + off + width)
            wsl = slice(off, off + width)
            ensure_group_tiles(groups)

            st_t = []
            for gi, g in enumerate(groups):
                t = stp.tile([P, NSL], bft, name=f"st_{j}_{pid}_{g}",
                             tag=f"st{gi}")
                nc.sync.dma_start(out=t[:, :width],
                                  in_=stb[g * P:(g + 1) * P, jsl])
                st_t.append(t)

            # PSUM: tags pc0/pc1/pg0/pg1 x bufs=2 = all 8 banks.  A 4-group
            # phase occupies both ring slots of each tag simultaneously.
            # Tiles are always full NSL wide (uniform pool slots); narrow
            # phases just use the leading [:, :width] columns.
            pc = [psp.tile([P, NSL], f32, name=f"pc_{j}_{pid}_{gi}",
                           tag=f"pc{gi % 2}")[:, :width]
                  for gi in range(len(groups))]
            pg = [psp.tile([P, NSL], f32, name=f"pg_{j}_{pid}_{gi}",
                           tag=f"pg{gi % 2}")[:, :width]
                  for gi in range(len(groups))]

            # fp8 DoubleRow x-side accumulation (k = 256 per MM)
            for k in range(K8):
                for gi, g in enumerate(groups):
                    xk = act_x[g][:, k]
                    nc.tensor.matmul(pc[gi], xk,
                                     wx_t["c"][(j, k)][:, :, wsl],
                                     start=(k == 0), stop=False, perf_mode=DR)
                    nc.tensor.matmul(pg[gi], xk,
                                     wx_t["g"][(j, k)][:, :, wsl],
                                     start=(k == 0), stop=False, perf_mode=DR)
            # fp8 DoubleRow state-side accumulation (first SCUT channels)
            for k in range(SK8):
                for gi, g in enumerate(groups):
                    sk = act_s8[g][:, k]
                    nc.tensor.matmul(pc[gi], sk,
                                     wsd_t["c"][(j, k)][:, :, wsl],
                                     start=False, stop=False, perf_mode=DR)
                    nc.tensor.matmul(pg[gi], sk,
                                     wsd_t["g"][(j, k)][:, :, wsl],
                                     start=False, stop=False, perf_mode=DR)
            # bf16 state-side accumulation (k = 128 per MM)
            for k in range(K16):
                q, kk = divmod(k, KQ)
                wc = ws_t[(j, "c", q)]
                wg = ws_t[(j, "g", q)]
                for gi, g in enumerate(groups):
                    sk = act_s[g][:, k, :]
                    nc.tensor.matmul(pc[gi], sk, wc[:, kk, wsl],
                                     start=False, stop=(k == K16 - 1))
                    nc.tensor.matmul(pg[gi], sk, wg[:, kk, wsl],
                                     start=False, stop=(k == K16 - 1))

            if mid_cb is not None:
                mid_cb()

            # epilogue for this (j, groups) slice
            for gi, g in enumerate(groups):
                sc = epp.tile([P, NSL], f32, name=f"sc_{j}_{pid}_{gi}",
                              tag="sc")
                sg = epp.tile([P, NSL], f32, name=f"sg_{j}_{pid}_{gi}",
                              tag="sg")
                if has_bc:
                    nc.vector.scalar_tensor_tensor(
                        sc[:, :width], pc[gi], RS, vt["bcb"][:, jsl],
                        op0=OP.mult, op1=OP.add)
                    nc.scalar.activation(sc[:, :width], sc[:, :width], AF.Tanh)
                else:
                    nc.scalar.activation(sc[:, :width], pc[gi], AF.Tanh,
                                         scale=RS)
                if has_bg:
                    nc.vector.scalar_tensor_tensor(
                        sg[:, :width], pg[gi], RS, vt["bgb"][:, jsl],
                        op0=OP.mult, op1=OP.add)
                    nc.scalar.activation(sg[:, :width], sg[:, :width], AF.Sigmoid)
                else:
                    nc.scalar.activation(sg[:, :width], pg[gi], AF.Sigmoid,
                                         scale=RS)

                # h = gc + alpha*(state - gc), with gc = gate*cand
                t2 = epp.tile([P, NSL], f32, name=f"t2_{j}_{pid}_{gi}",
                              tag="t2")
                nc.vector.tensor_mul(t2[:, :width], sc[:, :width], sg[:, :width])
                nc.vector.tensor_sub(sc[:, :width], st_t[gi][:, :width], t2[:, :width])
                nc.vector.tensor_mul(sc[:, :width], sc[:, :width], alpha_t[:, jsl])
                nc.vector.tensor_add(t2[:, :width], t2[:, :width], sc[:, :width])

                slot = j if stat_slot is None else stat_slot
                nc.vector.bn_stats(out=stats_t[g][:, slot, :], in_=t2[:, :width])
                nc.vector.tensor_copy(out=h_t[g][:, jsl], in_=t2[:, :width])

                if do_ln:
                    # layernorm + output for this group
                    mv = normp.tile([P, 2], f32, name=f"mv_{g}", tag="mv")
                    nc.vector.bn_aggr(out=mv[:], in_=stats_t[g][:])
                    rstd = normp.tile([P, 1], f32, name=f"rstd_{g}",
                                      tag="rstd")
                    nc.scalar.activation(rstd[:], mv[:, 1:2], AF.Sqrt,
                                         bias=eps_t[:])
                    nc.vector.reciprocal(rstd[:], rstd[:])
                    for q in range(NJ):
                        hs = slice(q * NSL, (q + 1) * NSL)
                        ot = outp.tile([P, NSL], f32,
                                       name=f"ot_{g}_{q}", tag="ot")
                        nc.vector.tensor_scalar(ot[:], h_t[g][:, hs],
                                                mv[:, 0:1], rstd[:],
                                                op0=OP.subtract, op1=OP.mult)
                        if has_gamma:
                            nc.vector.tensor_mul(ot[:], ot[:],
                                                 vt["gammab"][:, hs])
                        if has_beta:
                            nc.vector.tensor_add(ot[:], ot[:],
                                                 vt["betab"][:, hs])
                        nc.sync.dma_start(out=out[g * P:(g + 1) * P, hs],
                                          in_=ot[:])

        # ---- main loops: j = H slice (outer), batch groups inner.
        # First j slice runs 4-group phases (38us of matmul per phase)
        # to cover the initial weight-streaming burst; the last j slice
        # runs single-group phases, with the final group split into two
        # half-width phases, to shorten the post-matmul tail.
        for j in range(NJ):
            if j == 0:
                # alpha is consumed by this phase's epilogue, so its
                # emission must precede it in program order — but its DMA
                # must queue behind the critical weight stream: emit it
                # between the matmuls and the epilogue via mid_cb.
                phase(j, (0, 1, 2, 3), 0, mid_cb=emit_alpha)
                # deferred non-critical DMAs: queue them only now so they
                # don't dilute the first phase's weight-stream bandwidth
                for g in range(4, G):
                    load_x(g)
                    load_s8(g)
                    load_s(g)
                phase(j, (4, 5, 6, 7), 1)
                load_wx(1)
                load_wsd(1)
                load_ws(1)
            elif j < NJ - 1:
                for gp in range(GP):
                    phase(j, (2 * gp, 2 * gp + 1), gp)
                    if gp == 0:
                        load_wx(j + 1)
                        load_wsd(j + 1)
                        load_ws(j + 1)
            else:
                for g in range(G - 1):
                    phase(j, (g,), g, do_ln=True)
                HW = NSL // 2
                phase(j, (G - 1,), G - 1, off=0, width=HW, stat_slot=NJ - 1)
                phase(j, (G - 1,), G, off=HW, width=HW, stat_slot=NJ,
                      do_ln=True)

    nc.compile()
    return nc


def _get_compiled(flags):
    if flags not in _compiled:
        _compiled[flags] = _build(flags)
    return _compiled[flags]


def kernel(x_t, state, Wc, Uc, bc, Wg, Ug, bg, log_step, gamma, beta):
    global LAST_RESULTS
    from concourse import bass_utils

    x_t = np.asarray(x_t, np.float32)
    state = np.asarray(state, np.float32)
    Wc = np.asarray(Wc, np.float32)
    Uc = np.asarray(Uc, np.float32)
    Wg = np.asarray(Wg, np.float32)
    Ug = np.asarray(Ug, np.float32)
    bc = np.asarray(bc, np.float32)
    bg = np.asarray(bg, np.float32)
    log_step = np.asarray(log_step, np.float32)
    gamma = np.asarray(gamma, np.float32)
    beta = np.asarray(beta, np.float32)

    # fold the recurrent weights and pre-tile for the device
    def w8tile(w, nk):  # [j,k,p,i,n] = 256*W[k*256+i*128+p, j*512+n], e4m3
        a = np.clip(w * SW8, -240.0, 240.0).astype(e4m3)
        return np.ascontiguousarray(
            a.reshape(nk, 2, P, NJ, NSL).transpose(3, 0, 2, 1, 4))

    def ws16tile(w):  # [j,p,k,n] = 4096*W[k*128+p, j*512+n], bf16
        a = (w * SW16).astype(bf16)
        return np.ascontiguousarray(
            a.reshape(K16, P, NJ, NSL).transpose(2, 1, 0, 3))

    Wcs = Wc[IN:] + Uc
    Wgs = Wg[IN:] + Ug
    w_maps = {
        "wcx8": w8tile(Wc[:IN], K8),
        "wgx8": w8tile(Wg[:IN], K8),
        "wcs8": w8tile(Wcs[:SCUT], SK8),
        "wgs8": w8tile(Wgs[:SCUT], SK8),
        "wcs16": ws16tile(Wcs[SCUT:]),
        "wgs16": ws16tile(Wgs[SCUT:]),
    }
    logb = np.ascontiguousarray(
        np.broadcast_to(log_step.reshape(1, H), (P, H)))

    flags = (bool(bc.any()), bool(bg.any()),
             bool((gamma != 1.0).any()), bool(beta.any()))
    vec_maps = {}
    if flags[0]:
        vec_maps["bcb"] = np.ascontiguousarray(
            np.broadcast_to(bc.reshape(1, H), (P, H)))
    if flags[1]:
        vec_maps["bgb"] = np.ascontiguousarray(
            np.broadcast_to(bg.reshape(1, H), (P, H)))
    if flags[2]:
        vec_maps["gammab"] = np.ascontiguousarray(
            np.broadcast_to(gamma.reshape(1, H), (P, H)))
    if flags[3]:
        vec_maps["betab"] = np.ascontiguousarray(
            np.broadcast_to(beta.reshape(1, H), (P, H)))

    nc = _get_compiled(flags)

    # per-core activation shards, pre-tiled
    def a8tile(a, nk):  # [g,p,k,i,m] = 16*a[g*128+m, k*256+i*128+p], e4m3
        q = np.clip(a * SX, -240.0, 240.0).astype(e4m3)
        return np.ascontiguousarray(
            q.reshape(G, P, nk, 2, P).transpose(0, 4, 2, 3, 1))

    def s16tile(a):  # [g,p,k,m] = a[g*128+m, k*128+p], bf16
        return np.ascontiguousarray(
            a.astype(bf16).reshape(G, P, K16, P).transpose(0, 3, 2, 1))

    in_maps = []
    for c in range(NCORES):
        rows = slice(c * BC, (c + 1) * BC)
        sr = state[rows]
        m = {
            "x8": a8tile(x_t[rows], K8),
            "s8": a8tile(sr[:, :SCUT], SK8),
            "s16": s16tile(sr[:, SCUT:]),
            "stb": np.ascontiguousarray(sr.astype(bf16)),
            "logb": logb,
        }
        m.update(w_maps)
        m.update(vec_maps)
        in_maps.append(m)

    trace_kwargs = {}
    if TRACE:
        trace_kwargs["trace_cores"] = list(range(NCORES))
    res = bass_utils.run_bass_kernel_spmd(
        nc, in_maps, core_ids=list(range(NCORES)), trace=TRACE,
        **trace_kwargs)
    LAST_RESULTS = res
    return np.concatenate([res.results[c]["out"] for c in range(NCORES)],
                          axis=0)


# revision 25
# speedup vs baseline: 1.0327x; 1.0049x over previous
"""Trainium2 Bass kernel for nn_BrainRegion (liquid-gated recurrent cell).

Computes, for full inputs (B=8192, IN=H=2048):
    xin  = concat([x_t, state], -1)
    cand = tanh(xin @ Wc + state @ Uc + bc)
    gate = sigmoid(xin @ Wg + state @ Ug + bg)
    alpha = exp(-1/exp(log_step))
    h    = alpha * state + (1 - alpha) * gate * cand
    out  = layernorm(h) * gamma + beta

Strategy: data-parallel over batch across 8 NeuronCores (1024 rows/core),
weights replicated.  Algebraic fold: xin@Wc + state@Uc == x_t@Wc[:IN] +
state@(Wc[IN:] + Uc), which removes one third of the FLOPs.  Mixed
precision on the TensorEngine: the x_t-side matmuls and the first 512
state channels run in fp8 e4m3 with perf_mode=DoubleRow (256-deep
contraction per instruction, issuing at the same 216ns as a 512-wide
bf16 matmul = 2x throughput); the remaining state-side channels run in
bf16 (the folded state weights are 3x larger in variance, so full fp8
there would blow the error budget).  Scale folding keeps the shared
PSUM accumulation consistent: activations*16 and W*256 in fp8,
Ws*4096 in bf16 (all powers of two, exact); the epilogue activations
fold 1/4096 back in.  Loop order is j-outer / group-inner so every
weight byte is DMAed exactly once; startup DMAs are emitted in exact
first-phase consumption order; the last j slice runs single-group
phases to minimize the post-matmul tail.
"""

import sys

if "/opt/trn_rl_repo" not in sys.path:
    sys.path.insert(0, "/opt/trn_rl_repo")

import numpy as np
import ml_dtypes

B, IN, H = 8192, 2048, 2048
NCORES = 8
BC = B // NCORES      # rows per core (1024)
P = 128               # partitions
G = BC // P           # batch groups per core (8)
GP = G // 2           # group pairs (4)
NJ = 4                # H slices
NSL = H // NJ         # slice width (512)
K8 = H // 256         # fp8 DoubleRow k-chunks on the x side (8)
SK8 = 2               # state-side fp8 DoubleRow k-chunks (first 512 ch)
SCUT = SK8 * 256      # state channels handled in fp8 (512)
K16 = (H - SCUT) // 128   # bf16 state k-chunks (12)
KQ = 4                # bf16 k-chunks per quarter weight tile
NQ = K16 // KQ        # quarter tiles (3)
EPS = 1e-5
SX = 16.0             # fp8 activation scale
SW8 = 256.0           # fp8 weight scale
SW16 = 4096.0         # bf16 state-weight scale (= SX*SW8, exact pow2)
RS = 1.0 / 4096.0     # epilogue rescale

bf16 = ml_dtypes.bfloat16
e4m3 = ml_dtypes.float8_e4m3

# Set by test.py to collect a hardware profile.
TRACE = False
LAST_RESULTS = None

_compiled = {}


def _build(flags):
    """Trace + compile the SPMD device program. flags = (has_bc, has_bg,
    has_gamma, has_beta) selects optional elementwise passes."""
    from contextlib import ExitStack

    import concourse.bass as bass
    import concourse.tile as tile
    from concourse import bacc, mybir

    has_bc, has_bg, has_gamma, has_beta = flags
    f32 = mybir.dt.float32
    bft = mybir.dt.bfloat16
    f8 = mybir.dt.float8e4
    AF = mybir.ActivationFunctionType
    OP = mybir.AluOpType
    DR = mybir.MatmulPerfMode.DoubleRow

    nc = bacc.Bacc("TRN2", target_bir_lowering=False, debug=False,
                   num_devices=NCORES)

    # DRAM I/O. Everything is pre-arranged on host so every DMA below is
    # contiguous (partition dim first):
    #   x8:    [G, P, K8, 2, P]    e4m3, [g,p,k,i,m] = 16*x[g*128+m, k*256+i*128+p]
    #   s8:    [G, P, SK8, 2, P]   e4m3, same layout for state channels < SCUT
    #   s16:   [G, P, K16, P]      bf16, [g,p,k,m] = s[g*128+m, SCUT+k*128+p]
    #   w*x8:  [NJ, K8, P, 2, NSL] e4m3, [j,k,p,i,n] = 256*W[k*256+i*128+p, j*512+n]
    #   w*s8:  [NJ, SK8, P, 2, NSL] e4m3, same for state-weight rows < SCUT
    #   w*s16: [NJ, P, K16, NSL]   bf16, [j,p,k,n] = 4096*W[SCUT+k*128+p, j*512+n]
    x8 = nc.dram_tensor("x8", [G, P, K8, 2, P], f8, kind="ExternalInput").ap()
    s8 = nc.dram_tensor("s8", [G, P, SK8, 2, P], f8,
                        kind="ExternalInput").ap()
    s16 = nc.dram_tensor("s16", [G, P, K16, P], bft,
                         kind="ExternalInput").ap()
    stb = nc.dram_tensor("stb", [BC, H], bft, kind="ExternalInput").ap()
    wcx8 = nc.dram_tensor("wcx8", [NJ, K8, P, 2, NSL], f8,
                          kind="ExternalInput").ap()
    wgx8 = nc.dram_tensor("wgx8", [NJ, K8, P, 2, NSL], f8,
                          kind="ExternalInput").ap()
    wcs8 = nc.dram_tensor("wcs8", [NJ, SK8, P, 2, NSL], f8,
                          kind="ExternalInput").ap()
    wgs8 = nc.dram_tensor("wgs8", [NJ, SK8, P, 2, NSL], f8,
                          kind="ExternalInput").ap()
    wcs16 = nc.dram_tensor("wcs16", [NJ, P, K16, NSL], bft,
                           kind="ExternalInput").ap()
    wgs16 = nc.dram_tensor("wgs16", [NJ, P, K16, NSL], bft,
                           kind="ExternalInput").ap()
    logb = nc.dram_tensor("logb", [P, H], f32, kind="ExternalInput").ap()
    vecs = {}
    for name, used in (("bcb", has_bc), ("bgb", has_bg),
                       ("gammab", has_gamma), ("betab", has_beta)):
        if used:
            vecs[name] = nc.dram_tensor(name, [P, H], f32,
                                        kind="ExternalInput").ap()
    out = nc.dram_tensor("out", [BC, H], f32, kind="ExternalOutput").ap()

    with tile.TileContext(nc) as tc, ExitStack() as ctx:
        singles = ctx.enter_context(tc.tile_pool(name="singles", bufs=1))
        actp = ctx.enter_context(tc.tile_pool(name="actp", bufs=1))
        wxp = ctx.enter_context(tc.tile_pool(name="wxp", bufs=2))
        wsp = ctx.enter_context(tc.tile_pool(name="wsp", bufs=2))
        psp = ctx.enter_context(tc.tile_pool(name="psp", bufs=2, space="PSUM"))
        epp = ctx.enter_context(tc.tile_pool(name="epp", bufs=2))
        stp = ctx.enter_context(tc.tile_pool(name="stp", bufs=2))
        hp = ctx.enter_context(tc.tile_pool(name="hp", bufs=1))
        statp = ctx.enter_context(tc.tile_pool(name="statp", bufs=1))
        normp = ctx.enter_context(tc.tile_pool(name="normp", bufs=4))
        outp = ctx.enter_context(tc.tile_pool(name="outp", bufs=3))

        wx_t = {"c": {}, "g": {}}   # (j, k) -> [P, 2, NSL] f8, x side
        wsd_t = {"c": {}, "g": {}}  # (j, k) -> [P, 2, NSL] f8, state side
        ws_t = {}                   # (j, mat, q) -> [P, KQ, NSL] bf16
        act_x = {}
        act_s8 = {}
        act_s = {}

        def load_wx(j):
            for k in range(K8):
                for mat, dram in (("c", wcx8), ("g", wgx8)):
                    t = wxp.tile([P, 2, NSL], f8, name=f"wx{mat}_{j}_{k}",
                                 tag=f"wx{mat}_{k}")
                    nc.sync.dma_start(out=t[:], in_=dram[j, k])
                    wx_t[mat][(j, k)] = t

        def load_wsd(j):
            for k in range(SK8):
                for mat, dram in (("c", wcs8), ("g", wgs8)):
                    t = wxp.tile([P, 2, NSL], f8, name=f"wsd{mat}_{j}_{k}",
                                 tag=f"wsd{mat}_{k}")
                    nc.sync.dma_start(out=t[:], in_=dram[j, k])
                    wsd_t[mat][(j, k)] = t

        def load_ws(j):
            for q in range(NQ):
                for mat, dram in (("c", wcs16), ("g", wgs16)):
                    t = wsp.tile([P, KQ, NSL], bft,
                                 name=f"ws{mat}_{j}_{q}", tag=f"ws{mat}{q}")
                    nc.sync.dma_start(
                        out=t[:], in_=dram[j][:, q * KQ:(q + 1) * KQ, :])
                    ws_t[(j, mat, q)] = t

        def load_x(g):
            t = actp.tile([P, K8, 2, P], f8, name=f"x8_{g}", tag=f"x{g}")
            nc.sync.dma_start(out=t[:], in_=x8[g])
            act_x[g] = t

        def load_s8(g):
            t = actp.tile([P, SK8, 2, P], f8, name=f"s8_{g}", tag=f"d{g}")
            nc.sync.dma_start(out=t[:], in_=s8[g])
            act_s8[g] = t

        def load_s(g):
            t = actp.tile([P, K16, P], bft, name=f"s16_{g}", tag=f"s{g}")
            nc.sync.dma_start(out=t[:], in_=s16[g])
            act_s[g] = t

        # ---- startup DMAs: ONLY what the first (4-group) phase consumes,
        # ordered to match its consumption.  Everything else is deferred so
        # the proportional-share DMA queues don't starve the critical path.
        for g in range(4):
            load_x(g)
        load_wx(0)
        for g in range(4):
            load_s8(g)
        load_wsd(0)
        for g in range(4):
            load_s(g)
        load_ws(0)

        alpha_t = singles.tile([P, H], bft, name="alpha_t")

        def emit_alpha():
            # alpha = exp(-exp(-log_step)) as a bf16 [P, H] broadcast,
            # computed in NSL chunks through the outp ring.
            for q in range(NJ):
                qsl = slice(q * NSL, (q + 1) * NSL)
                t = outp.tile([P, NSL], f32, name=f"lg_{q}", tag="ot")
                nc.sync.dma_start(out=t[:], in_=logb[:, qsl])
                nc.scalar.activation(t[:], t[:], AF.Exp, scale=-1.0)
                nc.scalar.activation(alpha_t[:, qsl], t[:], AF.Exp,
                                     scale=-1.0)

        eps_t = singles.tile([P, 1], f32, name="eps_t")
        nc.vector.memset(eps_t[:], EPS)
        vt = {}
        for name in vecs:
            vt[name] = singles.tile([P, H], f32, name=name + "_t")
            nc.sync.dma_start(out=vt[name][:], in_=vecs[name][:])

        h_t = {}
        stats_t = {}

        def ensure_group_tiles(groups):
            for g in groups:
                if g not in h_t:
                    h_t[g] = hp.tile([P, H], bft, name=f"h_{g}", tag=f"h{g}")
                    nslot = NJ + 1 if g == G - 1 else NJ
                    stats_t[g] = statp.tile([P, nslot, 6], f32,
                                            name=f"stats_{g}", tag=f"st{g}")

        def phase(j, groups, pid, off=0, width=NSL, stat_slot=None,
                  do_ln=False, mid_cb=None):
            """One PSUM phase: H-slice j (columns [off, off+width) within
            the slice) accumulation + epilogue for the given batch groups."""
            jsl = slice(j * NSL + off, j * NSL + off + width)
            wsl = slice(off, off + width)
            ensure_group_tiles(groups)

            st_t = []
            for gi, g in enumerate(groups):
                t = stp.tile([P, NSL], bft, name=f"st_{j}_{pid}_{g}",
                             tag=f"st{gi}")
                nc.sync.dma_start(out=t[:, :width],
                                  in_=stb[g * P:(g + 1) * P, jsl])
                st_t.append(t)

            # PSUM: tags pc0/pc1/pg0/pg1 x bufs=2 = all 8 banks.  A 4-group
            # phase occupies both ring slots of each tag simultaneously.
            # Tiles are always full NSL wide (uniform pool slots); narrow
            # phases just use the leading [:, :width] columns.
            pc = [psp.tile([P, NSL], f32, name=f"pc_{j}_{pid}_{gi}",
                           tag=f"pc{gi % 2}")[:, :width]
                  for gi in range(len(groups))]
            pg = [psp.tile([P, NSL], f32, name=f"pg_{j}_{pid}_{gi}",
                           tag=f"pg{gi % 2}")[:, :width]
                  for gi in range(len(groups))]

            # fp8 DoubleRow x-side accumulation (k = 256 per MM)
            for k in range(K8):
                for gi, g in enumerate(groups):
                    xk = act_x[g][:, k]
                    nc.tensor.matmul(pc[gi], xk,
                                     wx_t["c"][(j, k)][:, :, wsl],
                                     start=(k == 0), stop=False, perf_mode=DR)
                    nc.tensor.matmul(pg[gi], xk,
                                     wx_t["g"][(j, k)][:, :, wsl],
                                     start=(k == 0), stop=False, perf_mode=DR)
            # fp8 DoubleRow state-side accumulation (first SCUT channels)
            for k in range(SK8):
                for gi, g in enumerate(groups):
                    sk = act_s8[g][:, k]
                    nc.tensor.matmul(pc[gi], sk,
                                     wsd_t["c"][(j, k)][:, :, wsl],
                                     start=False, stop=False, perf_mode=DR)
                    nc.tensor.matmul(pg[gi], sk,
                                     wsd_t["g"][(j, k)][:, :, wsl],
                                     start=False, stop=False, perf_mode=DR)
            # bf16 state-side accumulation (k = 128 per MM)
            for k in range(K16):
                q, kk = divmod(k, KQ)
                wc = ws_t[(j, "c", q)]
                wg = ws_t[(j, "g", q)]
                for gi, g in enumerate(groups):
                    sk = act_s[g][:, k, :]
                    nc.tensor.matmul(pc[gi], sk, wc[:, kk, wsl],
                                     start=False, stop=(k == K16 - 1))
                    nc.tensor.matmul(pg[gi], sk, wg[:, kk, wsl],
                                     start=False, stop=(k == K16 - 1))

            if mid_cb is not None:
                mid_cb()

            # epilogue for this (j, groups) slice
            for gi, g in enumerate(groups):
                sc = epp.tile([P, NSL], f32, name=f"sc_{j}_{pid}_{gi}",
                              tag="sc")
                sg = epp.tile([P, NSL], f32, name=f"sg_{j}_{pid}_{gi}",
                              tag="sg")
                if has_bc:
                    nc.vector.scalar_tensor_tensor(
                        sc[:, :width], pc[gi], RS, vt["bcb"][:, jsl],
                        op0=OP.mult, op1=OP.add)
                    nc.scalar.activation(sc[:, :width], sc[:, :width], AF.Tanh)
                else:
                    nc.scalar.activation(sc[:, :width], pc[gi], AF.Tanh,
                                         scale=RS)
                if has_bg:
                    nc.vector.scalar_tensor_tensor(
                        sg[:, :width], pg[gi], RS, vt["bgb"][:, jsl],
                        op0=OP.mult, op1=OP.add)
                    nc.scalar.activation(sg[:, :width], sg[:, :width], AF.Sigmoid)
                else:
                    # sigmoid(x) = 0.5 + 0.5*tanh(x/2): reusing the tanh
                    # table avoids the 1.3us ACT table swap per phase
                    nc.scalar.activation(sg[:, :width], pg[gi], AF.Tanh,
                                         scale=RS / 2)
                    nc.vector.tensor_scalar(sg[:, :width], sg[:, :width],
                                            0.5, 0.5, op0=OP.mult,
                                            op1=OP.add)

                # h = gc + alpha*(state - gc), with gc = gate*cand
                t2 = epp.tile([P, NSL], f32, name=f"t2_{j}_{pid}_{gi}",
                              tag="t2")
                nc.vector.tensor_mul(t2[:, :width], sc[:, :width], sg[:, :width])
                nc.vector.tensor_sub(sc[:, :width], st_t[gi][:, :width], t2[:, :width])
                nc.vector.tensor_mul(sc[:, :width], sc[:, :width], alpha_t[:, jsl])
                nc.vector.tensor_add(t2[:, :width], t2[:, :width], sc[:, :width])

                slot = j if stat_slot is None else stat_slot
                nc.vector.bn_stats(out=stats_t[g][:, slot, :], in_=t2[:, :width])
                nc.vector.tensor_copy(out=h_t[g][:, jsl], in_=t2[:, :width])

                if do_ln:
                    # layernorm + output for this group
                    mv = normp.tile([P, 2], f32, name=f"mv_{g}", tag="mv")
                    nc.vector.bn_aggr(out=mv[:], in_=stats_t[g][:])
                    rstd = normp.tile([P, 1], f32, name=f"rstd_{g}",
                                      tag="rstd")
                    nc.scalar.activation(rstd[:], mv[:, 1:2], AF.Sqrt,
                                         bias=eps_t[:])
                    nc.vector.reciprocal(rstd[:], rstd[:])
                    for q in range(NJ):
                        hs = slice(q * NSL, (q + 1) * NSL)
                        ot = outp.tile([P, NSL], f32,
                                       name=f"ot_{g}_{q}", tag="ot")
                        nc.vector.tensor_scalar(ot[:], h_t[g][:, hs],
                                                mv[:, 0:1], rstd[:],
                                                op0=OP.subtract, op1=OP.mult)
                        if has_gamma:
                            nc.vector.tensor_mul(ot[:], ot[:],
                                                 vt["gammab"][:, hs])
                        if has_beta:
                            nc.vector.tensor_add(ot[:], ot[:],
                                                 vt["betab"][:, hs])
                        nc.sync.dma_start(out=out[g * P:(g + 1) * P, hs],
                                          in_=ot[:])

        # ---- main loops: j = H slice (outer), batch groups inner.
        # First j slice runs 4-group phases (38us of matmul per phase)
        # to cover the initial weight-streaming burst; the last j slice
        # runs single-group phases, with the final group split into two
        # half-width phases, to shorten the post-matmul tail.
        for j in range(NJ):
            if j == 0:
                # alpha is consumed by this phase's epilogue, so its
                # emission must precede it in program order — but its DMA
                # must queue behind the critical weight stream: emit it
                # between the matmuls and the epilogue via mid_cb.
                phase(j, (0, 1, 2, 3), 0, mid_cb=emit_alpha)
                # deferred non-critical DMAs: queue them only now so they
                # don't dilute the first phase's weight-stream bandwidth
                for g in range(4, G):
                    load_x(g)
                    load_s8(g)
                    load_s(g)
                phase(j, (4, 5, 6, 7), 1)
                load_wx(1)
                load_wsd(1)
                load_ws(1)
            elif j < NJ - 1:
                for gp in range(GP):
                    phase(j, (2 * gp, 2 * gp + 1), gp)
                    if gp == 0:
                        load_wx(j + 1)
                        load_wsd(j + 1)
                        load_ws(j + 1)
            else:
                for g in range(G - 1):
                    phase(j, (g,), g, do_ln=True)
                HW = NSL // 2
                phase(j, (G - 1,), G - 1, off=0, width=HW, stat_slot=NJ - 1)
                phase(j, (G - 1,), G, off=HW, width=HW, stat_slot=NJ,
                      do_ln=True)

    nc.compile()
    return nc


def _get_compiled(flags):
    if flags not in _compiled:
        _compiled[flags] = _build(flags)
    return _compiled[flags]


def kernel(x_t, state, Wc, Uc, bc, Wg, Ug, bg, log_step, gamma, beta):
    global LAST_RESULTS
    from concourse import bass_utils

    x_t = np.asarray(x_t, np.float32)
    state = np.asarray(state, np.float32)
    Wc = np.asarray(Wc, np.float32)
    Uc = np.asarray(Uc, np.float32)
    Wg = np.asarray(Wg, np.float32)
    Ug = np.asarray(Ug, np.float32)
    bc = np.asarray(bc, np.float32)
    bg = np.asarray(bg, np.float32)
    log_step = np.asarray(log_step, np.float32)
    gamma = np.asarray(gamma, np.float32)
    beta = np.asarray(beta, np.float32)

    # fold the recurrent weights and pre-tile for the device
    def w8tile(w, nk):  # [j,k,p,i,n] = 256*W[k*256+i*128+p, j*512+n], e4m3
        a = np.clip(w * SW8, -240.0, 240.0).astype(e4m3)
        return np.ascontiguousarray(
            a.reshape(nk, 2, P, NJ, NSL).transpose(3, 0, 2, 1, 4))

    def ws16tile(w):  # [j,p,k,n] = 4096*W[k*128+p, j*512+n], bf16
        a = (w * SW16).astype(bf16)
        return np.ascontiguousarray(
            a.reshape(K16, P, NJ, NSL).transpose(2, 1, 0, 3))

    Wcs = Wc[IN:] + Uc
    Wgs = Wg[IN:] + Ug
    w_maps = {
        "wcx8": w8tile(Wc[:IN], K8),
        "wgx8": w8tile(Wg[:IN], K8),
        "wcs8": w8tile(Wcs[:SCUT], SK8),
        "wgs8": w8tile(Wgs[:SCUT], SK8),
        "wcs16": ws16tile(Wcs[SCUT:]),
        "wgs16": ws16tile(Wgs[SCUT:]),
    }
    logb = np.ascontiguousarray(
        np.broadcast_to(log_step.reshape(1, H), (P, H)))

    flags = (bool(bc.any()), bool(bg.any()),
             bool((gamma != 1.0).any()), bool(beta.any()))
    vec_maps = {}
    if flags[0]:
        vec_maps["bcb"] = np.ascontiguousarray(
            np.broadcast_to(bc.reshape(1, H), (P, H)))
    if flags[1]:
        vec_maps["bgb"] = np.ascontiguousarray(
            np.broadcast_to(bg.reshape(1, H), (P, H)))
    if flags[2]:
        vec_maps["gammab"] = np.ascontiguousarray(
            np.broadcast_to(gamma.reshape(1, H), (P, H)))
    if flags[3]:
        vec_maps["betab"] = np.ascontiguousarray(
            np.broadcast_to(beta.reshape(1, H), (P, H)))

    nc = _get_compiled(flags)

    # per-core activation shards, pre-tiled
    def a8tile(a, nk):  # [g,p,k,i,m] = 16*a[g*128+m, k*256+i*128+p], e4m3
        q = np.clip(a * SX, -240.0, 240.0).astype(e4m3)
        return np.ascontiguousarray(
            q.reshape(G, P, nk, 2, P).transpose(0, 4, 2, 3, 1))

    def s16tile(a):  # [g,p,k,m] = a[g*128+m, k*128+p], bf16
        return np.ascontiguousarray(
            a.astype(bf16).reshape(G, P, K16, P).transpose(0, 3, 2, 1))

    in_maps = []
    for c in range(NCORES):
        rows = slice(c * BC, (c + 1) * BC)
        sr = state[rows]
        m = {
            "x8": a8tile(x_t[rows], K8),
            "s8": a8tile(sr[:, :SCUT], SK8),
            "s16": s16tile(sr[:, SCUT:]),
            "stb": np.ascontiguousarray(sr.astype(bf16)),
            "logb": logb,
        }
        m.update(w_maps)
        m.update(vec_maps)
        in_maps.append(m)

    trace_kwargs = {}
    if TRACE:
        trace_kwargs["trace_cores"] = list(range(NCORES))
    res = bass_utils.run_bass_kernel_spmd(
        nc, in_maps, core_ids=list(range(NCORES)), trace=TRACE,
        **trace_kwargs)
    LAST_RESULTS = res
    return np.concatenate([res.results[c]["out"] for c in range(NCORES)],
                          axis=0)
